# revision 11
# baseline (speedup 1.0000x reference)
"""Trainium2 Bass kernel for BNBQuantizedLinear (group-quantized linear).

Computes y = x @ dequant(W)^T + bias with
  dequant(W)[o,i] = W[o,i]*scale[g] + wmin[g],   g = group of 128 along i,
  scale[g] = (max_g - min_g)/15.

Strategy (single fp16 pass — tolerance is 2e-2, fp16 gives ~4.4e-4):
  - Group-min folded into the dequantized weight (wd = w*scale + min), so
    the matmul is just y = x @ wd^T + b.  No Xbar term, no hi/lo split.
  - x uploaded pre-cast fp16, pre-tiled on host into per-s-tile slabs
    [i_in_ktile(part), ktile, s] so each slab is a stack of ready-made
    lhsT k-tiles (no PE transposes of x).
  - Weight fp16 [1376, 4096]; per 128-row block: DVE group min/max ->
    scale; dequant (w*scale+min) in place, split DVE/ACT/GpSimd; ONE
    XBAR dma_start_transpose moves the whole dequantized block into the
    resident wdT chunk tile [128 i, 32 k, cn o] (PE does zero transposes).
  - Warmup schedule: the first W s-tiles are processed per-chunk in chunk
    readiness order (chunk ci ready after its 3-4 weight blocks), so PE
    starts as soon as chunk0 is dequantized instead of waiting for all.
  - Steady state: 32 k x 3 chunk fp16 matmuls per s-tile (stationary =
    x k-tile, ldweights hidden under 1376 moving cols), DVE bias add,
    DMA out.

Sharding: tensor-parallel over out_features (11008 = 8*1376); x replicated.
"""

import numpy as np
from contextlib import ExitStack

import concourse.bass as bass
import concourse.tile as tile
import concourse.mybir as mb
from concourse import bass_utils

F32 = mb.dt.float32
F16 = mb.dt.float16

# Problem shapes (hardcoded per harness contract).
B, S, I, O = 4, 2048, 4096, 11008
N_CORES = 8
O_SH = O // N_CORES          # 1376 out features per core
GROUP = 128                  # quant group size along i
N_G = I // GROUP             # 32 groups per row
S_FLAT = B * S               # 8192
S_TILE = 128
N_ST = S_FLAT // S_TILE      # 64 s-tiles
K_T = I // 128               # 32 contraction tiles
O_BLK = 128
N_OB = (O_SH + O_BLK - 1) // O_BLK   # 11 blocks (last = 96 rows)
O_CHUNKS = [(0, 512), (512, 512), (1024, O_SH - 1024)]
CHUNK_OF_BLOCK = [0, 0, 0, 0, 1, 1, 1, 1, 2, 2, 2]

N_WARM = 5                   # s-tiles processed per-chunk during dequant
DQ_DVE = 0                   # dequant groups on DVE
DQ_ACT = 16                  # dequant groups on ACT (rest on GpSimd)


def _split_multi_waits(nc, max_waits=1):
    """This walrus build rejects >1 semaphore wait on a single instruction.
    Split: keep the last wait on the instruction, hoist the rest onto
    wait-only NoOps inserted immediately before it on the same engine."""
    n = 0
    for fn in nc.m.functions:
        for bb in fn.blocks:
            rebuilt, changed = [], False
            for inst in bb.instructions:
                si = getattr(inst, "sync_info", None)
                if si is not None and len(si.on_wait) > max_waits:
                    waits = list(si.on_wait)
                    for i, w in enumerate(waits[:-max_waits]):
                        ni = mb.InstNoOp(name=f"{inst.name}-wsplit{i}", ins=[], outs=[])
                        ni.engine = inst.engine
                        ni.sync_info = mb.SyncInfo(on_wait=[w], on_update=[])
                        nc.register_instruction(ni, overwrite=True)
                        rebuilt.append(ni)
                    inst.sync_info = mb.SyncInfo(
                        on_wait=waits[-max_waits:], on_update=list(si.on_update)
                    )
                    changed = True
                    n += 1
                rebuilt.append(inst)
            if changed:
                bb.instructions = rebuilt
    return n


def build_nc():
    nc = bass.Bass("TRN2", target_bir_lowering=False, debug=False,
                   enable_asserts=False)
    x_d = nc.dram_tensor("x", [S_FLAT, I], F16, kind="ExternalInput").ap()
    w_d = nc.dram_tensor("w", [O_SH, I], F16, kind="ExternalInput").ap()
    b_d = nc.dram_tensor("b", [O_SH], F32, kind="ExternalInput").ap()
    y_d = nc.dram_tensor("y", [S_FLAT, O_SH], F32, kind="ExternalOutput").ap()

    with tile.TileContext(nc) as tc:
        with ExitStack() as ctx:
            singles = ctx.enter_context(tc.tile_pool(name="singles", bufs=1))
            wpool = ctx.enter_context(tc.tile_pool(name="wpool", bufs=4))
            xpool = ctx.enter_context(tc.tile_pool(name="xpool", bufs=2))
            xhold = ctx.enter_context(tc.tile_pool(name="xhold", bufs=N_WARM))
            tpool = ctx.enter_context(tc.tile_pool(name="tpool", bufs=1))
            small = ctx.enter_context(tc.tile_pool(name="small", bufs=4))
            ysb_pool = ctx.enter_context(tc.tile_pool(name="ysb", bufs=2))
            ysw_pool = ctx.enter_context(tc.tile_pool(name="ysw", bufs=2))
            ps_y = ctx.enter_context(tc.tile_pool(name="ps_y", bufs=2, space="PSUM"))

            # bias replicated across partitions (SWDGE broadcast DMA)
            bias_rep = singles.tile([128, O_SH], F32)
            b_bc = bass.AP(tensor=b_d.tensor, offset=b_d.offset,
                           ap=[[0, 128]] + list(b_d.ap))
            nc.gpsimd.dma_start(out=bias_rep[:], in_=b_bc)

            # resident transposed dequantized weights, one tile per chunk:
            # wdT[ci] = [128 i-in-ktile, 32 ktile, cn o]
            wdT = [singles.tile([128, K_T, cn], F16, tag=f"wdT{ci}",
                                name=f"wdT{ci}")
                   for ci, (c0, cn) in enumerate(O_CHUNKS)]

            # warmup x slabs, loaded up-front and held
            xw = []
            for st in range(N_WARM):
                x_t = xhold.tile([128, I], F16, tag="xh", name=f"xw_{st}")
                nc.sync.dma_start(x_t[:], x_d[st * S_TILE:(st + 1) * S_TILE, :])
                xw.append(x_t)

            # ---- dequant of the weight shard (no PE involvement) ----
            for ob in range(N_OB):
                o0 = ob * O_BLK
                p = min(O_BLK, O_SH - o0)     # 128 or 96
                w_t = wpool.tile([128, I], F16, tag="w")
                nc.scalar.dma_start(w_t[:p], w_d[o0:o0 + p, :])
                w_g = w_t[:p].rearrange("p (g d) -> p g d", g=N_G)

                mn16 = small.tile([128, N_G], F16, tag="mn")
                mx16 = small.tile([128, N_G], F16, tag="mx")
                # fold 128-wide groups to 32 with two DVE TTs (2x mode),
                # then reduce (the reduce cannot run in the DVE 2x mode).
                for op, out16 in ((mb.AluOpType.min, mn16),
                                  (mb.AluOpType.max, mx16)):
                    t1 = tpool.tile([128, N_G, 64], F16, tag="t1")
                    nc.vector.tensor_tensor(out=t1[:p], in0=w_g[:, :, 0:64],
                                            in1=w_g[:, :, 64:128], op=op)
                    t2 = tpool.tile([128, N_G, 32], F16, tag="t2")
                    nc.vector.tensor_tensor(out=t2[:p], in0=t1[:p, :, 0:32],
                                            in1=t1[:p, :, 32:64], op=op)
                    nc.vector.tensor_reduce(out=out16[:p], in_=t2[:p],
                                            axis=mb.AxisListType.X, op=op)
                sc = small.tile([128, N_G], F32, tag="sc")
                mn = small.tile([128, N_G], F32, tag="mnf")
                nc.vector.tensor_tensor(out=sc[:p], in0=mx16[:p], in1=mn16[:p],
                                        op=mb.AluOpType.subtract)
                nc.vector.tensor_scalar_mul(sc[:p], sc[:p], 1.0 / 15.0)
                nc.vector.tensor_copy(out=mn[:p], in_=mn16[:p])

                # wd = w*scale + min, in place, split across DVE/ACT/GpSimd
                for g in range(N_G):
                    lo, hi = g * GROUP, (g + 1) * GROUP
                    if g < DQ_DVE:
                        nc.vector.tensor_scalar(
                            out=w_t[:p, lo:hi], in0=w_t[:p, lo:hi],
                            scalar1=sc[:p, g:g + 1], scalar2=mn[:p, g:g + 1],
                            op0=mb.AluOpType.mult, op1=mb.AluOpType.add)
                    elif g < DQ_DVE + DQ_ACT:
                        nc.scalar.activation(
                            out=w_t[:p, lo:hi], in_=w_t[:p, lo:hi],
                            func=mb.ActivationFunctionType.Identity,
                            bias=mn[:p, g:g + 1], scale=sc[:p, g:g + 1])
                    else:
                        nc.gpsimd.tensor_scalar(
                            out=w_t[:p, lo:hi], in0=w_t[:p, lo:hi],
                            scalar1=sc[:p, g:g + 1], scalar2=mn[:p, g:g + 1],
                            op0=mb.AluOpType.mult, op1=mb.AluOpType.add)

                # one XBAR DMA transposes the whole block into the chunk tile
                ci = CHUNK_OF_BLOCK[ob]
                cc0 = o0 - O_CHUNKS[ci][0]
                nc.sync.dma_start_transpose(wdT[ci][:, :, cc0:cc0 + p],
                                            w_t[:p, :])

            # ---- matmul sweeps ----
            def sweep(x_t, st, cis, y_pool):
                pys = {}
                for ci in cis:
                    c0, cn = O_CHUNKS[ci]
                    pys[ci] = ps_y.tile([128, cn], F32, tag=f"py{ci}",
                                        name=f"py_{st}_{ci}")
                for k in range(K_T):
                    lhs = x_t[:, k * 128:(k + 1) * 128]
                    for ci in cis:
                        c0, cn = O_CHUNKS[ci]
                        nc.tensor.matmul(pys[ci][:, :cn], lhs,
                                         wdT[ci][:, k, :cn],
                                         start=(k == 0), stop=(k == K_T - 1))
                if len(cis) == 3:
                    y_sb = y_pool.tile([128, O_SH], F32, tag="ysb")
                    for ci in cis:
                        c0, cn = O_CHUNKS[ci]
                        nc.vector.tensor_tensor(out=y_sb[:, c0:c0 + cn],
                                                in0=pys[ci][:, :cn],
                                                in1=bias_rep[:, c0:c0 + cn],
                                                op=mb.AluOpType.add)
                    nc.sync.dma_start(y_d[st * S_TILE:(st + 1) * S_TILE, :],
                                      y_sb[:])
                else:
                    ci, = cis
                    c0, cn = O_CHUNKS[ci]
                    y_sb = y_pool.tile([128, 512], F32, tag="ysw")
                    nc.vector.tensor_tensor(out=y_sb[:, :cn],
                                            in0=pys[ci][:, :cn],
                                            in1=bias_rep[:, c0:c0 + cn],
                                            op=mb.AluOpType.add)
                    nc.sync.dma_start(
                        y_d[st * S_TILE:(st + 1) * S_TILE, c0:c0 + cn],
                        y_sb[:, :cn])

            # warmup: first N_WARM s-tiles consumed per-chunk in readiness order
            for ci in range(len(O_CHUNKS)):
                for st in range(N_WARM):
                    sweep(xw[st], st, [ci], ysw_pool)

            # steady state
            xq = {}

            def prefetch(st):
                x_t = xpool.tile([128, I], F16, tag="x", name=f"x_{st}")
                nc.sync.dma_start(x_t[:], x_d[st * S_TILE:(st + 1) * S_TILE, :])
                xq[st] = x_t

            prefetch(N_WARM)
            if N_WARM + 1 < N_ST:
                prefetch(N_WARM + 1)
            for st in range(N_WARM, N_ST):
                if st + 2 < N_ST:
                    prefetch(st + 2)
                sweep(xq.pop(st), st, [0, 1, 2], ysb_pool)

    _split_multi_waits(nc)
    return nc


_NC_CACHE = None


def _get_nc():
    global _NC_CACHE
    if _NC_CACHE is None:
        _NC_CACHE = build_nc()
    return _NC_CACHE


last_run_info = {}


def kernel(x: np.ndarray, weight: np.ndarray, bias: np.ndarray) -> np.ndarray:
    assert x.shape == (B, S, I) and weight.shape == (O, I) and bias.shape == (O,)
    nc = _get_nc()

    # Host-side input marshaling: fp16 cast + per-s-tile k-major tiling of x
    # so each [128, 4096] DMA slab is a stack of ready-made lhsT k-tiles:
    # slab[st][p][k*128+s] = x[st*128+s, k*128+p].
    x16 = np.asarray(x, dtype=np.float16).reshape(S_FLAT, I)
    xt = np.ascontiguousarray(
        x16.reshape(N_ST, S_TILE, K_T, 128).transpose(0, 3, 2, 1)
    ).reshape(S_FLAT, I)
    w16 = np.asarray(weight, dtype=np.float16)
    bias = np.ascontiguousarray(np.asarray(bias, dtype=np.float32))

    in_maps = []
    for c in range(N_CORES):
        sl = slice(c * O_SH, (c + 1) * O_SH)
        in_maps.append({
            "x": xt,
            "w": np.ascontiguousarray(w16[sl]),
            "b": np.ascontiguousarray(bias[sl]),
        })

    res = bass_utils.run_bass_kernel_spmd(nc, in_maps, core_ids=list(range(N_CORES)))
    last_run_info["exec_time_ns"] = res.exec_time_ns
    y = np.concatenate([res.results[c]["y"] for c in range(N_CORES)], axis=1)
    return np.ascontiguousarray(y.reshape(B, S, O))


# revision 13
# speedup vs baseline: 1.0415x; 1.0415x over previous
"""Trainium2 Bass kernel for BNBQuantizedLinear (group-quantized linear).

Computes y = x @ dequant(W)^T + bias with
  dequant(W)[o,i] = W[o,i]*scale[g] + wmin[g],   g = group of 128 along i,
  scale[g] = (max_g - min_g)/15.

Strategy (single fp16 pass — tolerance is 2e-2, fp16 gives ~4.4e-4):
  - Group-min folded into the dequantized weight (wd = w*scale + min), so
    the matmul is just y = x @ wd^T + b.  No Xbar term, no hi/lo split.
  - x uploaded pre-cast fp16, pre-tiled on host into per-s-tile slabs
    [i_in_ktile(part), ktile, s] so each slab is a stack of ready-made
    lhsT k-tiles (no PE transposes of x).
  - Weight fp16 [1376, 4096]; per 128-row block: DVE group min/max ->
    scale; dequant (w*scale+min) in place, split DVE/ACT/GpSimd; ONE
    XBAR dma_start_transpose moves the whole dequantized block into the
    resident wdT chunk tile [128 i, 32 k, cn o] (PE does zero transposes).
  - Warmup schedule: the first W s-tiles are processed per-chunk in chunk
    readiness order (chunk ci ready after its 3-4 weight blocks), so PE
    starts as soon as chunk0 is dequantized instead of waiting for all.
  - Steady state: 32 k x 3 chunk fp16 matmuls per s-tile (stationary =
    x k-tile, ldweights hidden under 1376 moving cols), DVE bias add,
    DMA out.

Sharding: tensor-parallel over out_features (11008 = 8*1376); x replicated.
"""

import numpy as np
from contextlib import ExitStack

import concourse.bass as bass
import concourse.tile as tile
import concourse.mybir as mb
from concourse import bass_utils

F32 = mb.dt.float32
F16 = mb.dt.float16

# Problem shapes (hardcoded per harness contract).
B, S, I, O = 4, 2048, 4096, 11008
N_CORES = 8
O_SH = O // N_CORES          # 1376 out features per core
GROUP = 128                  # quant group size along i
N_G = I // GROUP             # 32 groups per row
S_FLAT = B * S               # 8192
S_TILE = 128
N_ST = S_FLAT // S_TILE      # 64 s-tiles
K_T = I // 128               # 32 contraction tiles
O_BLK = 128
N_OB = (O_SH + O_BLK - 1) // O_BLK   # 11 blocks (last = 96 rows)
O_CHUNKS = [(0, 512), (512, 512), (1024, O_SH - 1024)]
CHUNK_OF_BLOCK = [0, 0, 0, 0, 1, 1, 1, 1, 2, 2, 2]

N_WARM = 6                   # s-tiles processed per-chunk during dequant
DQ_DVE = 4                   # dequant groups on DVE
DQ_ACT = 16                  # dequant groups on ACT (rest on GpSimd)


def _split_multi_waits(nc, max_waits=1):
    """This walrus build rejects >1 semaphore wait on a single instruction.
    Split: keep the last wait on the instruction, hoist the rest onto
    wait-only NoOps inserted immediately before it on the same engine."""
    n = 0
    for fn in nc.m.functions:
        for bb in fn.blocks:
            rebuilt, changed = [], False
            for inst in bb.instructions:
                si = getattr(inst, "sync_info", None)
                if si is not None and len(si.on_wait) > max_waits:
                    waits = list(si.on_wait)
                    for i, w in enumerate(waits[:-max_waits]):
                        ni = mb.InstNoOp(name=f"{inst.name}-wsplit{i}", ins=[], outs=[])
                        ni.engine = inst.engine
                        ni.sync_info = mb.SyncInfo(on_wait=[w], on_update=[])
                        nc.register_instruction(ni, overwrite=True)
                        rebuilt.append(ni)
                    inst.sync_info = mb.SyncInfo(
                        on_wait=waits[-max_waits:], on_update=list(si.on_update)
                    )
                    changed = True
                    n += 1
                rebuilt.append(inst)
            if changed:
                bb.instructions = rebuilt
    return n


def build_nc():
    nc = bass.Bass("TRN2", target_bir_lowering=False, debug=False,
                   enable_asserts=False)
    x_d = nc.dram_tensor("x", [S_FLAT, I], F16, kind="ExternalInput").ap()
    w_d = nc.dram_tensor("w", [O_SH, I], F16, kind="ExternalInput").ap()
    b_d = nc.dram_tensor("b", [O_SH], F32, kind="ExternalInput").ap()
    y_d = nc.dram_tensor("y", [S_FLAT, O_SH], F32, kind="ExternalOutput").ap()

    with tile.TileContext(nc) as tc:
        with ExitStack() as ctx:
            singles = ctx.enter_context(tc.tile_pool(name="singles", bufs=1))
            wpool = ctx.enter_context(tc.tile_pool(name="wpool", bufs=3))
            xpool = ctx.enter_context(tc.tile_pool(name="xpool", bufs=2))
            xhold = ctx.enter_context(tc.tile_pool(name="xhold", bufs=N_WARM))
            tpool = ctx.enter_context(tc.tile_pool(name="tpool", bufs=1))
            small = ctx.enter_context(tc.tile_pool(name="small", bufs=4))
            ysb_pool = ctx.enter_context(tc.tile_pool(name="ysb", bufs=2))
            ysw_pool = ctx.enter_context(tc.tile_pool(name="ysw", bufs=2))
            ps_y = ctx.enter_context(tc.tile_pool(name="ps_y", bufs=2, space="PSUM"))

            # bias replicated across partitions (SWDGE broadcast DMA)
            bias_rep = singles.tile([128, O_SH], F32)
            b_bc = bass.AP(tensor=b_d.tensor, offset=b_d.offset,
                           ap=[[0, 128]] + list(b_d.ap))
            nc.gpsimd.dma_start(out=bias_rep[:], in_=b_bc)

            # resident transposed dequantized weights, one tile per chunk:
            # wdT[ci] = [128 i-in-ktile, 32 ktile, cn o]
            wdT = [singles.tile([128, K_T, cn], F16, tag=f"wdT{ci}",
                                name=f"wdT{ci}")
                   for ci, (c0, cn) in enumerate(O_CHUNKS)]

            # warmup x slabs, loaded up-front and held
            xw = []
            for st in range(N_WARM):
                x_t = xhold.tile([128, I], F16, tag="xh", name=f"xw_{st}")
                nc.sync.dma_start(x_t[:], x_d[st * S_TILE:(st + 1) * S_TILE, :])
                xw.append(x_t)

            # ---- dequant of the weight shard (no PE involvement) ----
            for ob in range(N_OB):
                o0 = ob * O_BLK
                p = min(O_BLK, O_SH - o0)     # 128 or 96
                w_t = wpool.tile([128, I], F16, tag="w")
                nc.scalar.dma_start(w_t[:p], w_d[o0:o0 + p, :])
                w_g = w_t[:p].rearrange("p (g d) -> p g d", g=N_G)

                mn16 = small.tile([128, N_G], F16, tag="mn")
                mx16 = small.tile([128, N_G], F16, tag="mx")
                # fold 128-wide groups to 32 with two DVE TTs (2x mode),
                # then reduce (the reduce cannot run in the DVE 2x mode).
                for op, out16 in ((mb.AluOpType.min, mn16),
                                  (mb.AluOpType.max, mx16)):
                    t1 = tpool.tile([128, N_G, 64], F16, tag="t1")
                    nc.vector.tensor_tensor(out=t1[:p], in0=w_g[:, :, 0:64],
                                            in1=w_g[:, :, 64:128], op=op)
                    t2 = tpool.tile([128, N_G, 32], F16, tag="t2")
                    nc.vector.tensor_tensor(out=t2[:p], in0=t1[:p, :, 0:32],
                                            in1=t1[:p, :, 32:64], op=op)
                    nc.vector.tensor_reduce(out=out16[:p], in_=t2[:p],
                                            axis=mb.AxisListType.X, op=op)
                sc = small.tile([128, N_G], F32, tag="sc")
                mn = small.tile([128, N_G], F32, tag="mnf")
                nc.vector.tensor_tensor(out=sc[:p], in0=mx16[:p], in1=mn16[:p],
                                        op=mb.AluOpType.subtract)
                nc.vector.tensor_scalar_mul(sc[:p], sc[:p], 1.0 / 15.0)
                nc.vector.tensor_copy(out=mn[:p], in_=mn16[:p])

                # wd = w*scale + min, in place, split across DVE/ACT/GpSimd
                for g in range(N_G):
                    lo, hi = g * GROUP, (g + 1) * GROUP
                    if g < DQ_DVE:
                        nc.vector.tensor_scalar(
                            out=w_t[:p, lo:hi], in0=w_t[:p, lo:hi],
                            scalar1=sc[:p, g:g + 1], scalar2=mn[:p, g:g + 1],
                            op0=mb.AluOpType.mult, op1=mb.AluOpType.add)
                    elif g < DQ_DVE + DQ_ACT:
                        nc.scalar.activation(
                            out=w_t[:p, lo:hi], in_=w_t[:p, lo:hi],
                            func=mb.ActivationFunctionType.Identity,
                            bias=mn[:p, g:g + 1], scale=sc[:p, g:g + 1])
                    else:
                        nc.gpsimd.tensor_scalar(
                            out=w_t[:p, lo:hi], in0=w_t[:p, lo:hi],
                            scalar1=sc[:p, g:g + 1], scalar2=mn[:p, g:g + 1],
                            op0=mb.AluOpType.mult, op1=mb.AluOpType.add)

                # one XBAR DMA transposes the whole block into the chunk tile
                ci = CHUNK_OF_BLOCK[ob]
                cc0 = o0 - O_CHUNKS[ci][0]
                nc.sync.dma_start_transpose(wdT[ci][:, :, cc0:cc0 + p],
                                            w_t[:p, :])

            # ---- matmul sweeps ----
            def sweep(x_t, st, cis, y_pool):
                pys = {}
                for ci in cis:
                    c0, cn = O_CHUNKS[ci]
                    pys[ci] = ps_y.tile([128, cn], F32, tag=f"py{ci}",
                                        name=f"py_{st}_{ci}")
                for k in range(K_T):
                    lhs = x_t[:, k * 128:(k + 1) * 128]
                    for ci in cis:
                        c0, cn = O_CHUNKS[ci]
                        nc.tensor.matmul(pys[ci][:, :cn], lhs,
                                         wdT[ci][:, k, :cn],
                                         start=(k == 0), stop=(k == K_T - 1))
                if len(cis) == 3:
                    y_sb = y_pool.tile([128, O_SH], F32, tag="ysb")
                    for ci in cis:
                        c0, cn = O_CHUNKS[ci]
                        nc.vector.tensor_tensor(out=y_sb[:, c0:c0 + cn],
                                                in0=pys[ci][:, :cn],
                                                in1=bias_rep[:, c0:c0 + cn],
                                                op=mb.AluOpType.add)
                    nc.sync.dma_start(y_d[st * S_TILE:(st + 1) * S_TILE, :],
                                      y_sb[:])
                else:
                    ci, = cis
                    c0, cn = O_CHUNKS[ci]
                    y_sb = y_pool.tile([128, 512], F32, tag="ysw")
                    nc.vector.tensor_tensor(out=y_sb[:, :cn],
                                            in0=pys[ci][:, :cn],
                                            in1=bias_rep[:, c0:c0 + cn],
                                            op=mb.AluOpType.add)
                    nc.sync.dma_start(
                        y_d[st * S_TILE:(st + 1) * S_TILE, c0:c0 + cn],
                        y_sb[:, :cn])

            # warmup: first N_WARM s-tiles consumed per-chunk in readiness order
            for ci in range(len(O_CHUNKS)):
                for st in range(N_WARM):
                    sweep(xw[st], st, [ci], ysw_pool)

            # steady state
            xq = {}

            def prefetch(st):
                x_t = xpool.tile([128, I], F16, tag="x", name=f"x_{st}")
                nc.sync.dma_start(x_t[:], x_d[st * S_TILE:(st + 1) * S_TILE, :])
                xq[st] = x_t

            prefetch(N_WARM)
            if N_WARM + 1 < N_ST:
                prefetch(N_WARM + 1)
            for st in range(N_WARM, N_ST):
                if st + 2 < N_ST:
                    prefetch(st + 2)
                sweep(xq.pop(st), st, [0, 1, 2], ysb_pool)

    _split_multi_waits(nc)
    return nc


_NC_CACHE = None


def _get_nc():
    global _NC_CACHE
    if _NC_CACHE is None:
        _NC_CACHE = build_nc()
    return _NC_CACHE


last_run_info = {}


def kernel(x: np.ndarray, weight: np.ndarray, bias: np.ndarray) -> np.ndarray:
    assert x.shape == (B, S, I) and weight.shape == (O, I) and bias.shape == (O,)
    nc = _get_nc()

    # Host-side input marshaling: fp16 cast + per-s-tile k-major tiling of x
    # so each [128, 4096] DMA slab is a stack of ready-made lhsT k-tiles:
    # slab[st][p][k*128+s] = x[st*128+s, k*128+p].
    x16 = np.asarray(x, dtype=np.float16).reshape(S_FLAT, I)
    xt = np.ascontiguousarray(
        x16.reshape(N_ST, S_TILE, K_T, 128).transpose(0, 3, 2, 1)
    ).reshape(S_FLAT, I)
    w16 = np.asarray(weight, dtype=np.float16)
    bias = np.ascontiguousarray(np.asarray(bias, dtype=np.float32))

    in_maps = []
    for c in range(N_CORES):
        sl = slice(c * O_SH, (c + 1) * O_SH)
        in_maps.append({
            "x": xt,
            "w": np.ascontiguousarray(w16[sl]),
            "b": np.ascontiguousarray(bias[sl]),
        })

    res = bass_utils.run_bass_kernel_spmd(nc, in_maps, core_ids=list(range(N_CORES)))
    last_run_info["exec_time_ns"] = res.exec_time_ns
    y = np.concatenate([res.results[c]["y"] for c in range(N_CORES)], axis=1)
    return np.ascontiguousarray(y.reshape(B, S, O))
